# revision 1
# baseline (speedup 1.0000x reference)
"""CTC batch loss on 8 TRN2 NeuronCores — pure data parallel, log-space DP.

Strategy (v4):
- Batch dim sharded 128 samples/core = SBUF partitions; free dim = the 129
  extended CTC states. Host pre-gathers emission log-probs lp[b,t,s] =
  log(y_pred[b,t,ext[b,s]] + eps) and ships them as bf16 (17 MB/core),
  plus tiny static mask tensors. All DP arithmetic runs on-device in f32
  log space (the alpha table needs ~177 nats of in-row dynamic range, so
  prob-space or bf16 state storage are mathematically impossible).
- Per step, exact logaddexp3 in batched-exp form: mx = max3;
  D = [a-mx | a1-mx | a2x-mx]; ONE ScalarE Exp over all three diffs;
  two bf16 vector adds; ONE ScalarE Ln; + emission add.
- The 511 sequential steps are split into a FORWARD chain (alpha, t=1..255)
  and an independent BACKWARD chain (beta, t=510..255, label-end injection
  via precomputed inj tensors), meeting at t*=255 with
  loss = -LSE_s(alpha_255 + beta_255). Two independent chains keep
  VectorE ~100% busy while each other's ScalarE/semaphore latency hides.
- Also monkeypatches around two toolchain bugs (see comments below):
  instructions with >1 sem waits and the Tile tail drain.
"""
import sys

for _p in ("/opt/trn_rl_repo", "/opt/pypackages"):
    if _p not in sys.path:
        sys.path.insert(0, _p)

import numpy as np
import ml_dtypes

import concourse.bass as bass
import concourse.tile as tile
from concourse import mybir
from concourse.bass_utils import run_bass_kernel_spmd

B, T, C, L = 1024, 512, 128, 64
S = 2 * L + 1          # 129 extended states
SP = 130               # padded state stride (even)
NCORES = 8
BL = B // NCORES       # 128 samples per core = SBUF partitions
EPS = 1e-7
NEG = -30000.0
CHUNK = 64             # t-steps per DMA chunk
NCHUNK = T // CHUNK

F32 = mybir.dt.float32
BF16 = mybir.dt.bfloat16
ALU = mybir.AluOpType
ACTF = mybir.ActivationFunctionType

# --- workaround: this walrus build rejects instructions with >2 sem waits
# ("Too many sync wait commands" in CoreV3 codegen). Tile's kernel-tail
# drain aggregates every outstanding token onto one SP Drain; split it
# into a chain of drains each carrying at most MAX_WAITS conditions.
_MAX_WAITS = 1


def _patched_drain_and_barrier(self, tick_clock, wait_clock):
    from concourse.vector_clock import ScopedClock

    drain_inst = self.nc.sync.drain()
    wait_clock.add_sem_waits(
        drain_inst.ins, ScopedClock({None: tick_clock.global_clock})
    )
    si = drain_inst.ins.sync_info
    waits = list(si.on_wait) if si and si.on_wait else []
    if len(waits) > _MAX_WAITS:
        drain_inst.ins.sync_info = mybir.SyncInfo(
            on_wait=waits[:_MAX_WAITS], on_update=list(si.on_update or [])
        )
        for i in range(_MAX_WAITS, len(waits), _MAX_WAITS):
            extra = self.nc.sync.drain()
            extra.ins.sync_info = mybir.SyncInfo(
                on_wait=waits[i:i + _MAX_WAITS], on_update=[]
            )

    self.nc.all_engine_barrier()
    assert self.sems is not None
    popped = self.nc._tile_sem_poison_stack.pop()
    assert popped is self._sem_poison
    self.nc.clear_and_free_semaphores(list(self.sems.allocated().values()))
    self.nc.all_engine_barrier()


tile.TileContext._drain_and_barrier = _patched_drain_and_barrier


# --- general BIR-level fix: split ANY instruction carrying more than one
# sem wait into single-wait Drain carriers + the original instruction with
# the last wait. Applied to the serialized BIR right before walrus.
def _split_multiwait_bir(ant_bir) -> bytes:
    import json as _json

    bir = _json.loads(ant_bir)
    n_split = 0
    for f in bir.get("functions", []):
        for blk in f.get("blocks", []):
            out = []
            for ins in blk.get("instructions", []):
                si = ins.get("sync_info")
                waits = (si or {}).get("on_wait") or []
                if len(waits) > 1:
                    for j, w in enumerate(waits[:-1]):
                        out.append({
                            "debug": ins.get("debug", 0),
                            "engine": ins["engine"],
                            "ins": [],
                            "name": f"{ins['name']}_w{j}",
                            "opcode": "Drain",
                            "outs": [],
                            "sync_info": {"on_update": [], "on_wait": [w]},
                        })
                    si["on_wait"] = [waits[-1]]
                    n_split += 1
                out.append(ins)
            blk["instructions"] = out
    return _json.dumps(bir).encode()


def _install_bir_splitter():
    import concourse.bass_utils as _bu
    import concourse.bass2jax as _b2j

    orig = _bu.compile_bir_kernel
    if getattr(orig, "_multiwait_patched", False):
        return

    def patched(ant_bir_str, compile_dir_path, neff_name="file.neff", **kw):
        return orig(_split_multiwait_bir(ant_bir_str), compile_dir_path,
                    neff_name=neff_name, **kw)

    patched._multiwait_patched = True
    _bu.compile_bir_kernel = patched
    if hasattr(_b2j, "compile_bir_kernel"):
        _b2j.compile_bir_kernel = patched


_install_bir_splitter()

# --- custom fused-LSE DVE op: out = max(x,y) + sq(relu(c0 + c1*(max-min)))
# i.e. logaddexp(x, y) with softplus(-t) ~ quadratic (validated e2e rel err 2e-3).
# Registered at runtime; sha computed on the fly.
USE_DVE_LSE = False
SP_C0 = 0.8129
SP_C1 = -0.2261
_LSE_OP = None


def _lse_ref(in0, in1, s0, s1, imm2):
    m = np.maximum(in0, in1)
    t = m - np.minimum(in0, in1)
    return (m + np.maximum(s0 + s1 * t, 0.0) ** 2).astype(np.float32)


def _make_lse_op():
    global _LSE_OP
    if _LSE_OP is not None:
        return _LSE_OP
    from concourse import dve_ops as dops
    from concourse.dve_spec import Spec, Src0, Src1, C0, C1, relu, sq, maxx, minn, lower
    from concourse.dve_spec import _has_src1
    from concourse.dve_uop import DveOpSpec

    name = "LSE_QSP_ANT"
    m = maxx(Src0, Src1)
    n = minn(Src0, Src1)
    body = m + sq(relu(C0 + C1 * (m - n)))
    spec = Spec(body=body, reference=_lse_ref)
    row = dops._CUSTOM_DVE_ROW_BASE + len(dops.OPS)
    shas = {}
    for ver in ("v3", "v4"):
        uops = lower(spec, ver=ver)
        tmp = DveOpSpec(name=name, opcode=row, uops=uops, rd1_en=_has_src1(spec))
        shas[ver] = tmp.sha(ver)
    op = dops.DveOp(name, spec, subdim=False, uops_sha=shas)
    dops.OPS.append(op)
    dops._SUB_OPCODE_FOR_NAME[name] = row
    dops.CUSTOM_DVE_SPECS[name] = spec
    _LSE_OP = op
    return op


_cached_nc = None


TSTAR = 255  # meet point: loss = -LSE_s(alpha[TSTAR] + beta[TSTAR])


def build_bass():
    nc = bass.Bass()
    lp_d = nc.declare_dram_parameter("lp", [BL, T * SP], BF16, isOutput=False)
    lsk_d = nc.declare_dram_parameter("lsk", [BL, SP], F32, isOutput=False)
    lskb_d = nc.declare_dram_parameter("lskb", [BL, SP], F32, isOutput=False)
    injr_d = nc.declare_dram_parameter("injr", [BL, 256 * SP], BF16, isOutput=False)
    inj511_d = nc.declare_dram_parameter("inj511", [BL, SP], F32, isOutput=False)
    out_d = nc.declare_dram_parameter("out", [BL, 1], F32, isOutput=True)

    with tile.TileContext(nc) as tc:
        with (
            tc.tile_pool(name="lpf", bufs=2) as lpf_pool,
            tc.tile_pool(name="lpb", bufs=2) as lpb_pool,
            tc.tile_pool(name="injp", bufs=2) as inj_pool,
            tc.tile_pool(name="persist", bufs=1) as pp,
        ):
            # forward state + scratch
            p_a = pp.tile([BL, S + 3], F32, tag="p_a")   # cols 0,1 pad NEG
            p_b = pp.tile([BL, S + 3], F32, tag="p_b")
            m1 = pp.tile([BL, SP], F32, tag="m1")
            a2x = pp.tile([BL, SP], F32, tag="a2x")
            mx = pp.tile([BL, SP], F32, tag="mx")
            dd = pp.tile([BL, 3 * SP], F32, tag="dd")
            ee = pp.tile([BL, 3 * SP], BF16, tag="ee")
            s01 = pp.tile([BL, SP], BF16, tag="s01")
            ssm = pp.tile([BL, SP], BF16, tag="ssm")
            lq = pp.tile([BL, SP], BF16, tag="lq")
            mlp = pp.tile([BL, SP], F32, tag="mlp")
            lsktile = pp.tile([BL, SP], F32, tag="lsktile")
            # backward state + scratch (fully separate so chains stay independent)
            zt = pp.tile([BL, S + 2], F32, tag="zt")     # cols S, S+1 pad NEG
            bt_a = pp.tile([BL, SP], F32, tag="bt_a")
            bt_b = pp.tile([BL, SP], F32, tag="bt_b")
            m1b = pp.tile([BL, SP], F32, tag="m1b")
            a2b = pp.tile([BL, SP], F32, tag="a2b")
            mxb = pp.tile([BL, SP], F32, tag="mxb")
            ddb = pp.tile([BL, 3 * SP], F32, tag="ddb")
            eeb = pp.tile([BL, 3 * SP], BF16, tag="eeb")
            s01b = pp.tile([BL, SP], BF16, tag="s01b")
            ssmb = pp.tile([BL, SP], BF16, tag="ssmb")
            lqb = pp.tile([BL, SP], BF16, tag="lqb")
            blb = pp.tile([BL, SP], F32, tag="blb")
            lskbtile = pp.tile([BL, SP], F32, tag="lskbtile")
            inj511tile = pp.tile([BL, SP], F32, tag="inj511tile")
            # readout
            am = pp.tile([BL, SP], F32, tag="am")
            mrow = pp.tile([BL, 1], F32, tag="mrow")
            nm = pp.tile([BL, 1], F32, tag="nm")
            erow = pp.tile([BL, SP], F32, tag="erow")
            ssum = pp.tile([BL, 1], F32, tag="ssum")
            lnr = pp.tile([BL, 1], F32, tag="lnr")
            loss = pp.tile([BL, 1], F32, tag="loss")

            nc.vector.memset(p_a[:, :], NEG)
            nc.vector.memset(p_b[:, :], NEG)
            nc.vector.memset(dd[:, :], 0.0)
            nc.vector.memset(zt[:, :], NEG)
            nc.vector.memset(bt_a[:, :], NEG)
            nc.vector.memset(bt_b[:, :], NEG)
            nc.vector.memset(ddb[:, :], 0.0)
            nc.sync.dma_start(out=lsktile[:, :], in_=lsk_d[:, :])
            nc.sync.dma_start(out=lskbtile[:, :], in_=lskb_d[:, :])
            nc.sync.dma_start(out=inj511tile[:, :], in_=inj511_d[:, :])

            pcur, pnew = p_a, p_b
            bcur, bnew = bt_a, bt_b
            binit_done = False
            for cblk in range(4):
                # fwd consumes lp chunk cblk (t = 64c..64c+63)
                lptf = lpf_pool.tile([BL, CHUNK * SP], BF16, tag="lpfc")
                lo = cblk * CHUNK * SP
                nc.sync.dma_start(out=lptf[:, :], in_=lp_d[:, lo:lo + CHUNK * SP])
                # bwd consumes lp chunk 7-cblk (t+1 = 511-i) and injr chunk cblk
                lptb = lpb_pool.tile([BL, CHUNK * SP], BF16, tag="lpbc")
                lob = (7 - cblk) * CHUNK * SP
                nc.sync.dma_start(out=lptb[:, :], in_=lp_d[:, lob:lob + CHUNK * SP])
                injt = inj_pool.tile([BL, CHUNK * SP], BF16, tag="injc")
                loi = cblk * CHUNK * SP
                nc.sync.dma_start(out=injt[:, :], in_=injr_d[:, loi:loi + CHUNK * SP])

                for il in range(CHUNK):
                    i = cblk * CHUNK + il
                    # ---- forward step t = i (i=0: init) ----
                    if i == 0:
                        nc.vector.tensor_copy(p_a[:, 2:4], lptf[:, 0:2])
                        nc.vector.tensor_max(bcur[:, 0:S], bt_b[:, 0:S],
                                             inj511tile[:, 0:S])
                        bnew = bt_b
                    else:
                        t = i
                        tl = il
                        lps = lptf[:, tl * SP: tl * SP + S]
                        a0 = pcur[:, 2:2 + S]
                        a1 = pcur[:, 1:1 + S]
                        a2 = pcur[:, 0:S]
                        nc.vector.tensor_max(m1[:, 0:S], a1, a0)
                        nc.vector.tensor_add(a2x[:, 0:S], a2, lsktile[:, 0:S])
                        nc.vector.tensor_max(mx[:, 0:S], m1[:, 0:S], a2x[:, 0:S])
                        nc.vector.tensor_sub(dd[:, 0:S], a0, mx[:, 0:S])
                        nc.vector.tensor_sub(dd[:, SP:SP + S], a1, mx[:, 0:S])
                        nc.vector.tensor_sub(dd[:, 2 * SP:2 * SP + S], a2x[:, 0:S],
                                             mx[:, 0:S])
                        nc.scalar.activation(ee[:, 0:3 * SP], dd[:, 0:3 * SP],
                                             ACTF.Exp)
                        nc.vector.tensor_add(s01[:, 0:SP], ee[:, 0:SP],
                                             ee[:, SP:2 * SP])
                        nc.vector.tensor_add(ssm[:, 0:SP], s01[:, 0:SP],
                                             ee[:, 2 * SP:3 * SP])
                        nc.scalar.activation(lq[:, 0:SP], ssm[:, 0:SP], ACTF.Ln)
                        nc.vector.tensor_add(mlp[:, 0:S], mx[:, 0:S], lps)
                        nc.vector.tensor_add(pnew[:, 2:2 + S], mlp[:, 0:S],
                                             lq[:, 0:S])
                        pcur, pnew = pnew, pcur

                    # ---- backward step t_b = 510 - i (uses lp[511-i], injr[i]) ----
                    tb1 = 511 - i          # = t_b + 1
                    tlb = tb1 - (7 - cblk) * CHUNK
                    lpsb = lptb[:, tlb * SP: tlb * SP + S]
                    injs = injt[:, il * SP: il * SP + S]
                    nc.vector.tensor_add(zt[:, 0:S], bcur[:, 0:S], lpsb)
                    z0 = zt[:, 0:S]
                    z1 = zt[:, 1:1 + S]
                    z2 = zt[:, 2:2 + S]
                    nc.vector.tensor_max(m1b[:, 0:S], z1, z0)
                    nc.vector.tensor_add(a2b[:, 0:S], z2, lskbtile[:, 0:S])
                    nc.vector.tensor_max(mxb[:, 0:S], m1b[:, 0:S], a2b[:, 0:S])
                    nc.vector.tensor_sub(ddb[:, 0:S], z0, mxb[:, 0:S])
                    nc.vector.tensor_sub(ddb[:, SP:SP + S], z1, mxb[:, 0:S])
                    nc.vector.tensor_sub(ddb[:, 2 * SP:2 * SP + S], a2b[:, 0:S],
                                         mxb[:, 0:S])
                    nc.scalar.activation(eeb[:, 0:3 * SP], ddb[:, 0:3 * SP],
                                         ACTF.Exp)
                    nc.vector.tensor_add(s01b[:, 0:SP], eeb[:, 0:SP],
                                         eeb[:, SP:2 * SP])
                    nc.vector.tensor_add(ssmb[:, 0:SP], s01b[:, 0:SP],
                                         eeb[:, 2 * SP:3 * SP])
                    nc.scalar.activation(lqb[:, 0:SP], ssmb[:, 0:SP], ACTF.Ln)
                    nc.vector.tensor_add(blb[:, 0:S], mxb[:, 0:S], lqb[:, 0:S])
                    nc.vector.tensor_max(bnew[:, 0:S], blb[:, 0:S], injs)
                    bcur, bnew = bnew, bcur

            # readout: loss = -LSE_s(alpha_255 + beta_255)
            nc.vector.tensor_add(am[:, 0:S], pcur[:, 2:2 + S], bcur[:, 0:S])
            nc.vector.tensor_reduce(out=mrow[:, 0:1], in_=am[:, 0:S],
                                    axis=mybir.AxisListType.X, op=ALU.max)
            nc.vector.tensor_scalar_mul(nm[:, 0:1], mrow[:, 0:1], -1.0)
            nc.scalar.activation(erow[:, 0:S], am[:, 0:S], ACTF.Exp,
                                 bias=nm[:, 0:1], scale=1.0)
            nc.vector.tensor_reduce(out=ssum[:, 0:1], in_=erow[:, 0:S],
                                    axis=mybir.AxisListType.X, op=ALU.add)
            nc.scalar.activation(lnr[:, 0:1], ssum[:, 0:1], ACTF.Ln)
            nc.vector.scalar_tensor_tensor(
                out=loss[:, 0:1], in0=mrow[:, 0:1], scalar=-1.0,
                in1=lnr[:, 0:1], op0=ALU.mult, op1=ALU.subtract)
            nc.sync.dma_start(out=out_d[:, :], in_=loss[:, 0:1])
    return nc


def _host_prep(y_pred, labels, input_length, label_length):
    blank = C - 1
    ext = np.full((B, S), blank, np.int32)
    ext[:, 1::2] = labels
    prev2 = np.concatenate([np.full((B, 2), -1, np.int32), ext[:, :-2]], axis=1)
    skip = (ext != blank) & (ext != prev2)                      # [B, S]

    q = np.take_along_axis(y_pred, ext[:, None, :], axis=2)     # [B, T, S]
    lp = np.log(q.astype(np.float32) + EPS)
    frozen = np.arange(T)[None, :] >= input_length[:, None]     # [B, T]
    lp[frozen, :] = 0.0

    lpp = np.zeros((B, T, SP), np.float32)
    lpp[:, :, :S] = lp
    lpp = lpp.reshape(B, T * SP).astype(ml_dtypes.bfloat16)

    lsk = np.where(skip, 0.0, NEG).astype(np.float32)           # [B, S]
    lskp = np.full((B, SP), NEG, np.float32)
    lskp[:, :S] = lsk
    lskbp = np.full((B, SP), NEG, np.float32)                   # lsk shifted by 2
    lskbp[:, :S - 2] = lsk[:, 2:]

    sellog = np.full((B, SP), NEG, np.float32)
    s_last = 2 * label_length.astype(np.int64)                  # [B]
    np.put_along_axis(sellog, s_last[:, None], 0.0, axis=1)
    np.put_along_axis(sellog, (s_last - 1)[:, None], 0.0, axis=1)

    # injr[b, j, :] = sellog[b] if input_length[b]-1 == 510-j else NEG, j=0..255
    lens = input_length.astype(np.int64)
    injr = np.full((B, 256, SP), NEG, np.float32)
    jsel = 510 - (lens - 1)                                     # j where injection lands
    has = (jsel >= 0) & (jsel <= 255)                           # len-1 in [255, 510]
    bi = np.nonzero(has)[0]
    injr[bi, jsel[bi], :] = sellog[bi, :]
    injr = injr.reshape(B, 256 * SP).astype(ml_dtypes.bfloat16)
    inj511 = np.where((lens - 1 == 511)[:, None], sellog,
                      NEG).astype(np.float32)                   # [B, SP]
    return lpp, lskp, lskbp, injr, inj511


def kernel(y_pred, labels, input_length, label_length):
    global _cached_nc
    lpp, lskp, lskbp, injr, inj511 = _host_prep(
        y_pred, labels, input_length, label_length)
    if _cached_nc is None:
        _cached_nc = build_bass()
    in_maps = []
    for i in range(NCORES):
        sl = slice(i * BL, (i + 1) * BL)
        in_maps.append({"lp": lpp[sl], "lsk": lskp[sl], "lskb": lskbp[sl],
                        "injr": injr[sl], "inj511": inj511[sl]})
    res = run_bass_kernel_spmd(_cached_nc, in_maps, list(range(NCORES)))
    out = np.concatenate([res.results[i]["out"] for i in range(NCORES)], axis=0)
    return out.astype(np.float32)



# revision 3
# speedup vs baseline: 2.8522x; 2.8522x over previous
"""CTC batch loss on 8 TRN2 NeuronCores — pure data parallel, log-space DP.

Strategy (v5):
- Batch dim sharded 128 samples/core = SBUF partitions. The 511 sequential
  DP steps are split into a forward alpha chain (t=0..255) and a backward
  beta chain (t=511..255) that MEET at t*=255; both chains live side by
  side in ONE 264-wide state row (fwd state at cols 2..130, bwd state
  REVERSED at cols 133..261), so every step is instructions over a single
  261-wide window covering both chains at once.
- Each LSE2 is one fused custom DVE op (quadratic-softplus approx):
      LSE_QSP(x, y) = max(x,y) + sq(relu(c0 + c1*(max-min)))
  (e2e rel err 2e-3 vs the 2e-2 gate). A second fused op folds the
  backward label-end injection AND the emission add into one instruction:
      INJLP(l2, lp; cinj) = max(l2, window0(Idx - cinj)) + lp
  where window0 yields 0.0 exactly on the 2-element inject window
  [cinj, cinj+1] and -3e38 elsewhere; cinj is a per-partition scalar
  streamed from a tiny [128, 256] table (9999 = no inject).
- Net: 4 DVE instructions per step, no ScalarE/act in the hot loop, no
  cross-engine syncs. Emission log-probs lp are host-gathered into the
  combined layout and shipped bf16 (17 MB/core).
- Also monkeypatches around two toolchain bugs (see comments below):
  instructions with >1 sem waits and the Tile tail drain.
"""
import sys

for _p in ("/opt/trn_rl_repo", "/opt/pypackages"):
    if _p not in sys.path:
        sys.path.insert(0, _p)

import numpy as np
import ml_dtypes

import concourse.bass as bass
import concourse.tile as tile
from concourse import mybir
from concourse.bass_utils import run_bass_kernel_spmd

B, T, C, L = 1024, 512, 128, 64
S = 2 * L + 1          # 129 extended states
NCORES = 8
BL = B // NCORES       # 128 samples per core = SBUF partitions
EPS = 1e-7
NEG = -30000.0

TW = 264               # combined state row width
FO = 2                 # fwd state s at col FO+s        (cols 2..130)
BO = 133               # bwd state s at col 261-s       (cols 133..261)
W = 261                # hot instruction window: cols [2, 263)
NSTEP = 256
CHUNK = 64             # t-steps per lp DMA chunk
NCHUNK = NSTEP // CHUNK
CINJ_OFF = 9999.0      # "no injection this step"

F32 = mybir.dt.float32
BF16 = mybir.dt.bfloat16
ALU = mybir.AluOpType
ACTF = mybir.ActivationFunctionType

SP_C0 = 0.8129
SP_C1 = -0.2261
INJ_BIG = -3.0e38

# --- workaround: this walrus build rejects instructions with >2 sem waits
# ("Too many sync wait commands" in CoreV3 codegen). Tile's kernel-tail
# drain aggregates every outstanding token onto one SP Drain; split it
# into a chain of drains each carrying at most MAX_WAITS conditions.
_MAX_WAITS = 1


def _patched_drain_and_barrier(self, tick_clock, wait_clock):
    from concourse.vector_clock import ScopedClock

    drain_inst = self.nc.sync.drain()
    wait_clock.add_sem_waits(
        drain_inst.ins, ScopedClock({None: tick_clock.global_clock})
    )
    si = drain_inst.ins.sync_info
    waits = list(si.on_wait) if si and si.on_wait else []
    if len(waits) > _MAX_WAITS:
        drain_inst.ins.sync_info = mybir.SyncInfo(
            on_wait=waits[:_MAX_WAITS], on_update=list(si.on_update or [])
        )
        for i in range(_MAX_WAITS, len(waits), _MAX_WAITS):
            extra = self.nc.sync.drain()
            extra.ins.sync_info = mybir.SyncInfo(
                on_wait=waits[i:i + _MAX_WAITS], on_update=[]
            )

    self.nc.all_engine_barrier()
    assert self.sems is not None
    popped = self.nc._tile_sem_poison_stack.pop()
    assert popped is self._sem_poison
    self.nc.clear_and_free_semaphores(list(self.sems.allocated().values()))
    self.nc.all_engine_barrier()


tile.TileContext._drain_and_barrier = _patched_drain_and_barrier


# --- general BIR-level fix: split ANY instruction carrying more than one
# sem wait into single-wait Drain carriers + the original instruction with
# the last wait. Applied to the serialized BIR right before walrus.
def _split_multiwait_bir(ant_bir) -> bytes:
    import json as _json

    bir = _json.loads(ant_bir)
    for f in bir.get("functions", []):
        for blk in f.get("blocks", []):
            out = []
            for ins in blk.get("instructions", []):
                si = ins.get("sync_info")
                waits = (si or {}).get("on_wait") or []
                if len(waits) > 1:
                    for j, w in enumerate(waits[:-1]):
                        out.append({
                            "debug": ins.get("debug", 0),
                            "engine": ins["engine"],
                            "ins": [],
                            "name": f"{ins['name']}_w{j}",
                            "opcode": "Drain",
                            "outs": [],
                            "sync_info": {"on_update": [], "on_wait": [w]},
                        })
                    si["on_wait"] = [waits[-1]]
                out.append(ins)
            blk["instructions"] = out
    return _json.dumps(bir).encode()


def _install_bir_splitter():
    import concourse.bass_utils as _bu
    import concourse.bass2jax as _b2j

    orig = _bu.compile_bir_kernel
    if getattr(orig, "_multiwait_patched", False):
        return

    def patched(ant_bir_str, compile_dir_path, neff_name="file.neff", **kw):
        return orig(_split_multiwait_bir(ant_bir_str), compile_dir_path,
                    neff_name=neff_name, **kw)

    patched._multiwait_patched = True
    _bu.compile_bir_kernel = patched
    if hasattr(_b2j, "compile_bir_kernel"):
        _b2j.compile_bir_kernel = patched


_install_bir_splitter()


# --- custom fused DVE ops, registered at runtime (shas computed on the fly).
def _lse_ref(in0, in1, s0, s1, imm2):
    a = np.asarray(in0, np.float32)
    b = np.asarray(in1, np.float32)
    m = np.maximum(a, b)
    t = m - np.minimum(a, b)
    return (m + np.maximum(s0 + s1 * t, 0.0) ** 2).astype(np.float32)


def _injlp_ref(in0, in1, s0, s1, imm2):
    a = np.asarray(in0, np.float32)
    lp = np.asarray(in1, np.float32)
    k = np.arange(a.shape[-1], dtype=np.float32)[None, :]
    u = k - (s0 if isinstance(s0, float) else np.asarray(s0, np.float32))
    p = u * (u - 1.0)
    inj = np.minimum(p, 1.0) * imm2
    return (np.maximum(a, inj) + lp).astype(np.float32)


_OPS = None


def _make_ops():
    global _OPS
    if _OPS is not None:
        return _OPS
    from concourse import dve_ops as dops
    from concourse.dve_spec import (Spec, Src0, Src1, C0, C1, One,
                                    relu, sq, maxx, minn, lower)
    from concourse.dve_spec import _has_src1
    from concourse.dve_uop import DveOpSpec

    def register(name, body, ref):
        spec = Spec(body=body, reference=ref)
        row = dops._CUSTOM_DVE_ROW_BASE + len(dops.OPS)
        shas = {}
        for ver in ("v3", "v4"):
            uops = lower(spec, ver=ver)
            tmp = DveOpSpec(name=name, opcode=row, uops=uops,
                            rd1_en=_has_src1(spec))
            shas[ver] = tmp.sha(ver)
        op = dops.DveOp(name, spec, subdim=False, uops_sha=shas)
        dops.OPS.append(op)
        dops._SUB_OPCODE_FOR_NAME[name] = row
        dops.CUSTOM_DVE_SPECS[name] = spec
        return op

    m = maxx(Src0, Src1)
    n = minn(Src0, Src1)
    lse_body = m + sq(relu(C0 + C1 * (m - n)))
    lse_op = register("LSE_QSP_ANT", lse_body, _lse_ref)

    from concourse.dve_spec import Idx, C2
    u = Idx - C0
    p = u * (u - One)
    inj_body = maxx(Src0, minn(p, One) * C2) + Src1
    inj_op = register("INJLP_ANT", inj_body, _injlp_ref)

    _OPS = (lse_op, inj_op)
    return _OPS


_cached_nc = None


def build_bass():
    lse_op, inj_op = _make_ops()
    nc = bass.Bass()
    lp_d = nc.declare_dram_parameter("lp", [BL, NSTEP * TW], BF16, isOutput=False)
    lsk_d = nc.declare_dram_parameter("lsk", [BL, TW], F32, isOutput=False)
    x0_d = nc.declare_dram_parameter("x0", [BL, TW], F32, isOutput=False)
    cl_d = nc.declare_dram_parameter("cl", [BL, NSTEP], F32, isOutput=False)
    out_d = nc.declare_dram_parameter("out", [BL, 1], F32, isOutput=True)

    with tile.TileContext(nc) as tc:
        with (
            tc.tile_pool(name="lpp", bufs=2) as lp_pool,
            tc.tile_pool(name="persist", bufs=1) as pp,
        ):
            x_a = pp.tile([BL, TW], F32, tag="x_a")
            x_b = pp.tile([BL, TW], F32, tag="x_b")
            a2x = pp.tile([BL, TW], F32, tag="a2x")
            l1 = pp.tile([BL, TW], F32, tag="l1")
            l2 = pp.tile([BL, TW], F32, tag="l2")
            lskt = pp.tile([BL, TW], F32, tag="lskt")
            clt = pp.tile([BL, NSTEP], F32, tag="clt")
            # readout
            am = pp.tile([BL, S], F32, tag="am")
            mrow = pp.tile([BL, 1], F32, tag="mrow")
            nm = pp.tile([BL, 1], F32, tag="nm")
            erow = pp.tile([BL, S], F32, tag="erow")
            ssum = pp.tile([BL, 1], F32, tag="ssum")
            lnr = pp.tile([BL, 1], F32, tag="lnr")
            loss = pp.tile([BL, 1], F32, tag="loss")

            nc.vector.memset(x_b[:, :], NEG)
            nc.sync.dma_start(out=x_a[:, :], in_=x0_d[:, :])
            nc.sync.dma_start(out=lskt[:, :], in_=lsk_d[:, :])
            nc.sync.dma_start(out=clt[:, :], in_=cl_d[:, :])

            xc, xn = x_a, x_b
            for cblk in range(NCHUNK):
                lpt = lp_pool.tile([BL, CHUNK * TW], BF16, tag="lpc")
                lo = cblk * CHUNK * TW
                nc.sync.dma_start(out=lpt[:, :], in_=lp_d[:, lo:lo + CHUNK * TW])
                for il in range(CHUNK):
                    i = cblk * CHUNK + il
                    nc.vector.tensor_add(a2x[:, 2:2 + W], xc[:, 0:W],
                                         lskt[:, 2:2 + W])
                    nc.vector._custom_dve(lse_op, out=l1[:, 2:2 + W],
                                          in0=xc[:, 2:2 + W], in1=xc[:, 1:1 + W],
                                          s0=SP_C0, s1=SP_C1)
                    nc.vector._custom_dve(lse_op, out=l2[:, 2:2 + W],
                                          in0=l1[:, 2:2 + W], in1=a2x[:, 2:2 + W],
                                          s0=SP_C0, s1=SP_C1)
                    nc.vector._custom_dve(inj_op, out=xn[:, 2:2 + W],
                                          in0=l2[:, 2:2 + W],
                                          in1=lpt[:, il * TW + 2: il * TW + 2 + W],
                                          s0=clt[:, i:i + 1], s1=0.0, imm2=INJ_BIG)
                    xc, xn = xn, xc

            # readout: loss = -LSE_s(alpha_255[s] + beta_255[s])
            # alpha at cols 2..130 (s=0..128), beta at cols 261..133 (reversed)
            nc.vector.tensor_add(am[:, 0:S], xc[:, FO:FO + S],
                                 xc[:, 261:132:-1])
            nc.vector.tensor_reduce(out=mrow[:, 0:1], in_=am[:, 0:S],
                                    axis=mybir.AxisListType.X, op=ALU.max)
            nc.vector.tensor_scalar_mul(nm[:, 0:1], mrow[:, 0:1], -1.0)
            nc.scalar.activation(erow[:, 0:S], am[:, 0:S], ACTF.Exp,
                                 bias=nm[:, 0:1], scale=1.0)
            nc.vector.tensor_reduce(out=ssum[:, 0:1], in_=erow[:, 0:S],
                                    axis=mybir.AxisListType.X, op=ALU.add)
            nc.scalar.activation(lnr[:, 0:1], ssum[:, 0:1], ACTF.Ln)
            nc.vector.scalar_tensor_tensor(
                out=loss[:, 0:1], in0=mrow[:, 0:1], scalar=-1.0,
                in1=lnr[:, 0:1], op0=ALU.mult, op1=ALU.subtract)
            nc.sync.dma_start(out=out_d[:, :], in_=loss[:, 0:1])
    # Raw Bass skips the InstISA byte-encoding pass (Bacc.compile runs it);
    # without it the NEFF compiler sees empty .instr -> "ISA wrong length".
    mybir.codegen_inst_isa_subclasses(nc)
    return nc


def _host_prep(y_pred, labels, input_length, label_length):
    blank = C - 1
    ext = np.full((B, S), blank, np.int32)
    ext[:, 1::2] = labels
    prev2 = np.concatenate([np.full((B, 2), -1, np.int32), ext[:, :-2]], axis=1)
    skip = (ext != blank) & (ext != prev2)                      # [B, S]

    q = np.take_along_axis(y_pred, ext[:, None, :], axis=2)     # [B, T, S]
    lp = np.log(q.astype(np.float32) + EPS)
    frozen = np.arange(T)[None, :] >= input_length[:, None]     # [B, T]
    lp[frozen, :] = 0.0

    lsk = np.where(skip, 0.0, NEG).astype(np.float32)           # [B, S]

    sellog = np.full((B, S), NEG, np.float32)
    s_last = 2 * label_length.astype(np.int64)                  # [B]
    np.put_along_axis(sellog, s_last[:, None], 0.0, axis=1)
    np.put_along_axis(sellog, (s_last - 1)[:, None], 0.0, axis=1)
    lens = input_length.astype(np.int64)

    # combined lp stream: fwd lp[i] at cols 2..130, bwd lp[510-i] reversed
    # at cols 133..261 (i=255 bwd part = 0 so the final bwd step yields
    # beta_255 without an lp add); NEG elsewhere so pad cols keep sinking.
    lpc = np.full((B, NSTEP, TW), NEG, np.float32)
    lpc[:, :, FO:FO + S] = lp[:, 0:NSTEP, :]
    lpc[:, 0:NSTEP - 1, BO:BO + S] = lp[:, 510:255:-1, ::-1]
    lpc[:, NSTEP - 1, BO:BO + S] = 0.0
    lpc = lpc.reshape(B, NSTEP * TW).astype(ml_dtypes.bfloat16)

    # combined skip gate: a2x[c] = X[c-2] + lskC[c]
    lskc = np.full((B, TW), NEG, np.float32)
    lskc[:, FO:FO + S] = lsk                                    # fwd: lsk[c-2]
    lskc[:, 135:262] = lsk[:, 2:S][:, ::-1]                     # bwd: lsk[263-c]

    # initial state: fwd alpha seed (0 at s=0), bwd g_511 = sellog + lp_511
    # for len==512 samples (reversed layout), NEG elsewhere.
    x0 = np.full((B, TW), NEG, np.float32)
    x0[:, FO] = 0.0
    g511 = np.where((lens == 512)[:, None], sellog + lp[:, 511, :], NEG)
    x0[:, BO:BO + S] = g511[:, ::-1].astype(np.float32)

    # injection column table: at step i = 511-len, window base Idx =
    # (col of s_last) - 2 = 259 - s_last; 9999 = no injection.
    cl = np.full((B, NSTEP), CINJ_OFF, np.float32)
    ii = 511 - lens                                             # [B]
    has = (ii >= 0) & (ii <= 255)
    bi = np.nonzero(has)[0]
    cl[bi, ii[bi]] = (259 - s_last[bi]).astype(np.float32)

    return lpc, lskc, x0, cl


def kernel(y_pred, labels, input_length, label_length):
    global _cached_nc
    lpc, lskc, x0, cl = _host_prep(y_pred, labels, input_length, label_length)
    if _cached_nc is None:
        _cached_nc = build_bass()
    in_maps = []
    for i in range(NCORES):
        sl = slice(i * BL, (i + 1) * BL)
        in_maps.append({"lp": lpc[sl], "lsk": lskc[sl], "x0": x0[sl],
                        "cl": cl[sl]})
    res = run_bass_kernel_spmd(_cached_nc, in_maps, list(range(NCORES)))
    out = np.concatenate([res.results[i]["out"] for i in range(NCORES)], axis=0)
    return out.astype(np.float32)


# revision 6
# speedup vs baseline: 2.9694x; 1.0411x over previous
"""CTC batch loss on 8 TRN2 NeuronCores — pure data parallel, log-space DP.

Strategy (v5):
- Batch dim sharded 128 samples/core = SBUF partitions. The 511 sequential
  DP steps are split into a forward alpha chain (t=0..255) and a backward
  beta chain (t=511..255) that MEET at t*=255; both chains live side by
  side in ONE 264-wide state row (fwd state at cols 2..130, bwd state
  REVERSED at cols 133..261), so every step is instructions over a single
  261-wide window covering both chains at once.
- Each LSE2 is one fused custom DVE op (quadratic-softplus approx):
      LSE_QSP(x, y) = max(x,y) + sq(relu(c0 + c1*(max-min)))
  (e2e rel err 2e-3 vs the 2e-2 gate). A second fused op folds the
  backward label-end injection AND the emission add into one instruction:
      INJLP(l2, lp; cinj) = max(l2, window0(Idx - cinj)) + lp
  where window0 yields 0.0 exactly on the 2-element inject window
  [cinj, cinj+1] and -3e38 elsewhere; cinj is a per-partition scalar
  streamed from a tiny [128, 256] table (9999 = no inject).
- Net: 4 DVE instructions per step, no ScalarE/act in the hot loop, no
  cross-engine syncs. Emission log-probs lp are host-gathered into the
  combined layout and shipped bf16 (17 MB/core).
- Also monkeypatches around two toolchain bugs (see comments below):
  instructions with >1 sem waits and the Tile tail drain.
"""
import sys

for _p in ("/opt/trn_rl_repo", "/opt/pypackages"):
    if _p not in sys.path:
        sys.path.insert(0, _p)

import numpy as np
import ml_dtypes

import concourse.bass as bass
import concourse.tile as tile
from concourse import mybir
from concourse.bass_utils import run_bass_kernel_spmd

B, T, C, L = 1024, 512, 128, 64
S = 2 * L + 1          # 129 extended states
NCORES = 8
BL = B // NCORES       # 128 samples per core = SBUF partitions
EPS = 1e-7
NEG = -30000.0

TW = 264               # combined state row width
FO = 2                 # fwd state s at col FO+s        (cols 2..130)
BO = 133               # bwd state s at col 261-s       (cols 133..261)
W = 261                # hot instruction window: cols [2, 263)
NSTEP = 256
# graduated lp chunk sizes (steps): small first chunks so step 0 starts
# ~2us in; all DMAs are issued upfront and arrive ahead of consumption.
CHUNKS = [4, 12, 16, 32, 32, 32, 32, 32, 32, 32]
assert sum(CHUNKS) == NSTEP
CINJ_OFF = 9999.0      # "no injection this step"

F32 = mybir.dt.float32
BF16 = mybir.dt.bfloat16
ALU = mybir.AluOpType
ACTF = mybir.ActivationFunctionType

SP_C0 = 0.8129
SP_C1 = -0.2261
INJ_BIG = -3.0e38

# --- workaround: this walrus build rejects instructions with >2 sem waits
# ("Too many sync wait commands" in CoreV3 codegen). Tile's kernel-tail
# drain aggregates every outstanding token onto one SP Drain; split it
# into a chain of drains each carrying at most MAX_WAITS conditions.
_MAX_WAITS = 1


def _patched_drain_and_barrier(self, tick_clock, wait_clock):
    from concourse.vector_clock import ScopedClock

    drain_inst = self.nc.sync.drain()
    wait_clock.add_sem_waits(
        drain_inst.ins, ScopedClock({None: tick_clock.global_clock})
    )
    si = drain_inst.ins.sync_info
    waits = list(si.on_wait) if si and si.on_wait else []
    if len(waits) > _MAX_WAITS:
        drain_inst.ins.sync_info = mybir.SyncInfo(
            on_wait=waits[:_MAX_WAITS], on_update=list(si.on_update or [])
        )
        for i in range(_MAX_WAITS, len(waits), _MAX_WAITS):
            extra = self.nc.sync.drain()
            extra.ins.sync_info = mybir.SyncInfo(
                on_wait=waits[i:i + _MAX_WAITS], on_update=[]
            )

    self.nc.all_engine_barrier()
    assert self.sems is not None
    popped = self.nc._tile_sem_poison_stack.pop()
    assert popped is self._sem_poison
    self.nc.clear_and_free_semaphores(list(self.sems.allocated().values()))
    self.nc.all_engine_barrier()


tile.TileContext._drain_and_barrier = _patched_drain_and_barrier


# --- general BIR-level fix: split ANY instruction carrying more than one
# sem wait into single-wait Drain carriers + the original instruction with
# the last wait. Applied to the serialized BIR right before walrus.
def _split_multiwait_bir(ant_bir) -> bytes:
    import json as _json

    bir = _json.loads(ant_bir)
    for f in bir.get("functions", []):
        for blk in f.get("blocks", []):
            out = []
            for ins in blk.get("instructions", []):
                si = ins.get("sync_info")
                waits = (si or {}).get("on_wait") or []
                if len(waits) > 1:
                    for j, w in enumerate(waits[:-1]):
                        out.append({
                            "debug": ins.get("debug", 0),
                            "engine": ins["engine"],
                            "ins": [],
                            "name": f"{ins['name']}_w{j}",
                            "opcode": "Drain",
                            "outs": [],
                            "sync_info": {"on_update": [], "on_wait": [w]},
                        })
                    si["on_wait"] = [waits[-1]]
                out.append(ins)
            blk["instructions"] = out
    return _json.dumps(bir).encode()


def _install_bir_splitter():
    import concourse.bass_utils as _bu
    import concourse.bass2jax as _b2j

    orig = _bu.compile_bir_kernel
    if getattr(orig, "_multiwait_patched", False):
        return

    def patched(ant_bir_str, compile_dir_path, neff_name="file.neff", **kw):
        return orig(_split_multiwait_bir(ant_bir_str), compile_dir_path,
                    neff_name=neff_name, **kw)

    patched._multiwait_patched = True
    _bu.compile_bir_kernel = patched
    if hasattr(_b2j, "compile_bir_kernel"):
        _b2j.compile_bir_kernel = patched


_install_bir_splitter()


# --- custom fused DVE ops, registered at runtime (shas computed on the fly).
def _lse_ref(in0, in1, s0, s1, imm2):
    a = np.asarray(in0, np.float32)
    b = np.asarray(in1, np.float32)
    m = np.maximum(a, b)
    t = m - np.minimum(a, b)
    return (m + np.maximum(s0 + s1 * t, 0.0) ** 2).astype(np.float32)


def _injlp_ref(in0, in1, s0, s1, imm2):
    a = np.asarray(in0, np.float32)
    lp = np.asarray(in1, np.float32)
    k = np.arange(a.shape[-1], dtype=np.float32)[None, :]
    u = k - (s0 if isinstance(s0, float) else np.asarray(s0, np.float32))
    p = u * (u - 1.0)
    inj = np.minimum(p, 1.0) * imm2
    return (np.maximum(a, inj) + lp).astype(np.float32)


_OPS = None


def _make_ops():
    global _OPS
    if _OPS is not None:
        return _OPS
    from concourse import dve_ops as dops
    from concourse.dve_spec import (Spec, Src0, Src1, C0, C1, One,
                                    relu, sq, maxx, minn, lower)
    from concourse.dve_spec import _has_src1
    from concourse.dve_uop import DveOpSpec

    def register(name, body, ref):
        spec = Spec(body=body, reference=ref)
        row = dops._CUSTOM_DVE_ROW_BASE + len(dops.OPS)
        shas = {}
        for ver in ("v3", "v4"):
            uops = lower(spec, ver=ver)
            tmp = DveOpSpec(name=name, opcode=row, uops=uops,
                            rd1_en=_has_src1(spec))
            shas[ver] = tmp.sha(ver)
        op = dops.DveOp(name, spec, subdim=False, uops_sha=shas)
        dops.OPS.append(op)
        dops._SUB_OPCODE_FOR_NAME[name] = row
        dops.CUSTOM_DVE_SPECS[name] = spec
        return op

    m = maxx(Src0, Src1)
    n = minn(Src0, Src1)
    lse_body = m + sq(relu(C0 + C1 * (m - n)))
    lse_op = register("LSE_QSP_ANT", lse_body, _lse_ref)

    from concourse.dve_spec import Idx, C2
    u = Idx - C0
    p = u * (u - One)
    inj_body = maxx(Src0, minn(p, One) * C2) + Src1
    inj_op = register("INJLP_ANT", inj_body, _injlp_ref)

    _OPS = (lse_op, inj_op)
    return _OPS


_cached_nc = None


def build_bass():
    lse_op, inj_op = _make_ops()
    nc = bass.Bass()
    lp_d = nc.declare_dram_parameter("lp", [BL, NSTEP * TW], BF16, isOutput=False)
    lsk_d = nc.declare_dram_parameter("lsk", [BL, TW], F32, isOutput=False)
    x0_d = nc.declare_dram_parameter("x0", [BL, TW], F32, isOutput=False)
    cl_d = nc.declare_dram_parameter("cl", [BL, NSTEP], F32, isOutput=False)
    out_d = nc.declare_dram_parameter("out", [BL, 1], F32, isOutput=True)

    with tile.TileContext(nc) as tc:
        with (
            tc.tile_pool(name="lpp", bufs=1) as lp_pool,
            tc.tile_pool(name="persist", bufs=1) as pp,
        ):
            x_a = pp.tile([BL, TW], F32, tag="x_a")
            x_b = pp.tile([BL, TW], F32, tag="x_b")
            a2x = pp.tile([BL, TW], F32, tag="a2x")
            l1 = pp.tile([BL, TW], F32, tag="l1")
            l2 = pp.tile([BL, TW], F32, tag="l2")
            lskt = pp.tile([BL, TW], F32, tag="lskt")
            clt = pp.tile([BL, NSTEP], F32, tag="clt")
            # readout scratch (NEG-padded QSP LSE tree)
            am = pp.tile([BL, 136], F32, tag="am")
            sc = pp.tile([BL, 176], F32, tag="sc")
            loss = pp.tile([BL, 1], F32, tag="loss")

            nc.vector.memset(x_b[:, :], NEG)
            nc.vector.memset(am[:, :], NEG)
            nc.vector.memset(sc[:, :], NEG)
            nc.sync.dma_start(out=x_a[:, :], in_=x0_d[:, :])
            nc.sync.dma_start(out=lskt[:, :], in_=lsk_d[:, :])
            nc.sync.dma_start(out=clt[:, :], in_=cl_d[:, :])
            # all lp chunks issued upfront; arrivals stay ahead of the loop
            lpts = []
            lo = 0
            for ci, csz in enumerate(CHUNKS):
                lpt = lp_pool.tile([BL, csz * TW], BF16, tag=f"lp{ci}")
                nc.sync.dma_start(out=lpt[:, :],
                                  in_=lp_d[:, lo * TW:(lo + csz) * TW])
                lpts.append((lpt, lo, csz))
                lo += csz

            xc, xn = x_a, x_b
            for lpt, lo, csz in lpts:
                for il in range(csz):
                    i = lo + il
                    nc.vector.tensor_add(a2x[:, 2:2 + W], xc[:, 0:W],
                                         lskt[:, 2:2 + W])
                    nc.vector._custom_dve(lse_op, out=l1[:, 2:2 + W],
                                          in0=xc[:, 2:2 + W], in1=xc[:, 1:1 + W],
                                          s0=SP_C0, s1=SP_C1)
                    nc.vector._custom_dve(lse_op, out=l2[:, 2:2 + W],
                                          in0=l1[:, 2:2 + W], in1=a2x[:, 2:2 + W],
                                          s0=SP_C0, s1=SP_C1)
                    nc.vector._custom_dve(inj_op, out=xn[:, 2:2 + W],
                                          in0=l2[:, 2:2 + W],
                                          in1=lpt[:, il * TW + 2: il * TW + 2 + W],
                                          s0=clt[:, i:i + 1], s1=0.0, imm2=INJ_BIG)
                    xc, xn = xn, xc

            # readout: loss = -LSE_s(alpha_255[s] + beta_255[s])
            # alpha at cols 2..130 (s=0..128), beta at cols 261..133 (reversed).
            # LSE over 129 values as a NEG-padded binary tree of QSP ops
            # (widths 129-65-33-17-9-5-3-2-1), all on DVE: no act tables.
            nc.vector.tensor_add(am[:, 0:S], xc[:, FO:FO + S],
                                 xc[:, 261:132:-1])

            def tree(op, out_t, out_o, in_t, in_o, wlo, whi):
                nc.vector._custom_dve(
                    op, out=out_t[:, out_o:out_o + wlo],
                    in0=in_t[:, in_o:in_o + wlo],
                    in1=in_t[:, in_o + wlo:in_o + wlo + wlo],
                    s0=SP_C0, s1=SP_C1)

            tree(lse_op, sc, 0, am, 0, 65, 129)     # 129 -> 65   (am[129]=NEG)
            tree(lse_op, sc, 80, sc, 0, 33, 65)     # 65  -> 33   (sc[65]=NEG)
            tree(lse_op, sc, 120, sc, 80, 17, 33)   # 33  -> 17   (sc[113]=NEG)
            tree(lse_op, sc, 140, sc, 120, 9, 17)   # 17  -> 9    (sc[137]=NEG)
            tree(lse_op, sc, 152, sc, 140, 5, 9)    # 9   -> 5    (sc[149]=NEG)
            tree(lse_op, sc, 160, sc, 152, 3, 5)    # 5   -> 3    (sc[157]=NEG)
            tree(lse_op, sc, 168, sc, 160, 2, 3)    # 3   -> 2    (sc[163]=NEG)
            tree(lse_op, sc, 172, sc, 168, 1, 2)    # 2   -> 1
            nc.vector.tensor_scalar_mul(loss[:, 0:1], sc[:, 172:173], -1.0)
            nc.sync.dma_start(out=out_d[:, :], in_=loss[:, 0:1])
    # Raw Bass skips the InstISA byte-encoding pass (Bacc.compile runs it);
    # without it the NEFF compiler sees empty .instr -> "ISA wrong length".
    mybir.codegen_inst_isa_subclasses(nc)
    return nc


def _host_prep(y_pred, labels, input_length, label_length):
    blank = C - 1
    ext = np.full((B, S), blank, np.int32)
    ext[:, 1::2] = labels
    prev2 = np.concatenate([np.full((B, 2), -1, np.int32), ext[:, :-2]], axis=1)
    skip = (ext != blank) & (ext != prev2)                      # [B, S]

    q = np.take_along_axis(y_pred, ext[:, None, :], axis=2)     # [B, T, S]
    lp = np.log(q.astype(np.float32) + EPS)
    frozen = np.arange(T)[None, :] >= input_length[:, None]     # [B, T]
    lp[frozen, :] = 0.0

    lsk = np.where(skip, 0.0, NEG).astype(np.float32)           # [B, S]

    sellog = np.full((B, S), NEG, np.float32)
    s_last = 2 * label_length.astype(np.int64)                  # [B]
    np.put_along_axis(sellog, s_last[:, None], 0.0, axis=1)
    np.put_along_axis(sellog, (s_last - 1)[:, None], 0.0, axis=1)
    lens = input_length.astype(np.int64)

    # combined lp stream: fwd lp[i] at cols 2..130, bwd lp[510-i] reversed
    # at cols 133..261 (i=255 bwd part = 0 so the final bwd step yields
    # beta_255 without an lp add); NEG elsewhere so pad cols keep sinking.
    lpc = np.full((B, NSTEP, TW), NEG, np.float32)
    lpc[:, :, FO:FO + S] = lp[:, 0:NSTEP, :]
    lpc[:, 0:NSTEP - 1, BO:BO + S] = lp[:, 510:255:-1, ::-1]
    lpc[:, NSTEP - 1, BO:BO + S] = 0.0
    lpc = lpc.reshape(B, NSTEP * TW).astype(ml_dtypes.bfloat16)

    # combined skip gate: a2x[c] = X[c-2] + lskC[c]
    lskc = np.full((B, TW), NEG, np.float32)
    lskc[:, FO:FO + S] = lsk                                    # fwd: lsk[c-2]
    lskc[:, 135:262] = lsk[:, 2:S][:, ::-1]                     # bwd: lsk[263-c]

    # initial state: fwd alpha seed (0 at s=0), bwd g_511 = sellog + lp_511
    # for len==512 samples (reversed layout), NEG elsewhere.
    x0 = np.full((B, TW), NEG, np.float32)
    x0[:, FO] = 0.0
    g511 = np.where((lens == 512)[:, None], sellog + lp[:, 511, :], NEG)
    x0[:, BO:BO + S] = g511[:, ::-1].astype(np.float32)

    # injection column table: at step i = 511-len, window base Idx =
    # (col of s_last) - 2 = 259 - s_last; 9999 = no injection.
    cl = np.full((B, NSTEP), CINJ_OFF, np.float32)
    ii = 511 - lens                                             # [B]
    has = (ii >= 0) & (ii <= 255)
    bi = np.nonzero(has)[0]
    cl[bi, ii[bi]] = (259 - s_last[bi]).astype(np.float32)

    return lpc, lskc, x0, cl


def kernel(y_pred, labels, input_length, label_length):
    global _cached_nc
    lpc, lskc, x0, cl = _host_prep(y_pred, labels, input_length, label_length)
    if _cached_nc is None:
        _cached_nc = build_bass()
    in_maps = []
    for i in range(NCORES):
        sl = slice(i * BL, (i + 1) * BL)
        in_maps.append({"lp": lpc[sl], "lsk": lskc[sl], "x0": x0[sl],
                        "cl": cl[sl]})
    res = run_bass_kernel_spmd(_cached_nc, in_maps, list(range(NCORES)))
    out = np.concatenate([res.results[i]["out"] for i in range(NCORES)], axis=0)
    return out.astype(np.float32)
